# revision 13
# baseline (speedup 1.0000x reference)
"""Trainium2 Bass kernel for BasicQuantumAttention.

Contract: kernel(**inputs) takes the FULL (unsharded) numpy inputs of the
reference problem (B=4, L=2048, D=512) and returns the full output
(out_real, out_imag), each [B, L, D] float32.

Sharding: 8 NeuronCores; core c handles batch b=c//2, query half h=c%2
(1024 queries). Each core computes the fused QKV projection only for its
own 1024 rows; the key/value projections are then exchanged within the
core pair via an in-kernel pairwise AllGather, so no projection work is
duplicated. Key order is global (rows 0:2047 of the batch) on both cores
of a pair, which keeps the SPMD program identical on every core.

Layouts (all matmuls bf16, f32 PSUM accumulation):
  - x is passed transposed per core: xT [6D=3072, own 1024 rows].
  - q,k projections are computed weight-stationary into ^T layout
    [feat, row]; v is computed into row-major [row, feat] layout.
  - Only the qkv output blocks the reference actually uses are computed
    (q_real, k_real, v_real, v_imag) -- 2/3 of the fused projection.
  - scores^T [key, query] = (k^T tile).T @ q^T; the per-key padding mask
    and the 1/sqrt(D) scale fold into the ACT Exp (bias/scale).
  - attn^T tiles directly feed Z^T accumulation; an M=1 ones-matmul
    produces the softmax row sums; normalization is deferred to after
    the output projection (diag scaling commutes with the right-matmul),
    and b_out is added in the same fused DVE op.
"""

import numpy as np
import ml_dtypes

B, L, D = 4, 2048, 512
P = 128
IN_F = 6 * D          # 3072 input features of the fused projection
QK_F = 2 * D          # selected output features: q_real block + k_real block
V_F = 2 * D           # selected output features: v_real block + v_imag block
KT = IN_F // P        # 24 contraction tiles
NCORES = 8
HALF = L // 2         # 1024 rows owned per core
SCALE = float(D) ** -0.5
NEG = -30000.0        # additive key mask (exp underflows to exactly 0)
GROUPS = [[0, 1], [2, 3], [4, 5], [6, 7]]
NSEND = 12            # blocks of [128, HALF] sent to the pair: 4 k^T + 8 v

_NC_CACHE = {}


def _build_program(reps=1):
    import concourse.bass as bass
    import concourse.bacc as bacc
    import concourse.mybir as mybir
    import concourse.tile as tile
    from contextlib import ExitStack

    f32 = mybir.dt.float32
    bf16 = mybir.dt.bfloat16
    AF = mybir.ActivationFunctionType
    ALU = mybir.AluOpType
    PSUM = bass.MemorySpace.PSUM

    nc = bacc.Bacc(
        "TRN2",
        debug=False,
        enable_asserts=False,
        target_bir_lowering=False,
        num_devices=NCORES,
    )

    xT_d = nc.dram_tensor("xT", [IN_F, HALF], bf16, kind="ExternalInput").ap()
    wqk_d = nc.dram_tensor("wqkT", [IN_F, QK_F], bf16, kind="ExternalInput").ap()
    wv_d = nc.dram_tensor("wvT", [IN_F, V_F], bf16, kind="ExternalInput").ap()
    wo_d = nc.dram_tensor("woT", [V_F, V_F], bf16, kind="ExternalInput").ap()
    mb_d = nc.dram_tensor("maskb", [P, L // P], f32, kind="ExternalInput").ap()
    bqk_d = nc.dram_tensor("bqk", [P, QK_F // P], f32, kind="ExternalInput").ap()
    bv_d = nc.dram_tensor("bv", [1, V_F], f32, kind="ExternalInput").ap()
    bo_d = nc.dram_tensor("bo", [1, V_F], f32, kind="ExternalInput").ap()
    y_d = nc.dram_tensor("y", [HALF, V_F], f32, kind="ExternalOutput").ap()

    k_send = nc.dram_tensor("k_send", [4 * P, HALF], bf16).ap()
    k_recv = nc.dram_tensor("k_recv", [8 * P, HALF], bf16).ap()
    v_send = nc.dram_tensor("v_send", [8 * P, HALF], bf16).ap()
    v_recv = nc.dram_tensor("v_recv", [16 * P, HALF], bf16).ap()

    xT_r = xT_d.rearrange("(t p) n -> t p n", p=P)
    wqk_r = wqk_d.rearrange("(t p) n -> t p n", p=P)
    wv_r = wv_d.rearrange("(t p) n -> t p n", p=P)
    wo_r = wo_d.rearrange("(t p) n -> t p n", p=P)
    ksend_r = k_send.rearrange("(i p) n -> i p n", p=P)
    krecv_r = k_recv.rearrange("(i p) n -> i p n", p=P)
    vsend_r = v_send.rearrange("(i p) n -> i p n", p=P)
    vrecv_r = v_recv.rearrange("(i p) n -> i p n", p=P)

    def _emit_body(tc, ctx):
        const = ctx.enter_context(tc.tile_pool(name="const", bufs=1))
        persist = ctx.enter_context(tc.tile_pool(name="persist", bufs=1))

        mb = const.tile([P, L // P], f32, tag="mb")
        nc.sync.dma_start(mb, mb_d)
        bqk = const.tile([P, QK_F // P], f32, tag="bqk")
        nc.sync.dma_start(bqk, bqk_d)
        bv = const.tile([1, V_F], f32, tag="bv")
        nc.sync.dma_start(bv, bv_d)
        bo = const.tile([1, V_F], f32, tag="bo")
        nc.sync.dma_start(bo, bo_d)
        ones_r = const.tile([1, P], f32, tag="ones_r")
        nc.vector.memset(ones_r, 1.0)
        ones_c = const.tile([P, 1], bf16, tag="ones_c")
        nc.vector.memset(ones_c, 1.0)
        ident1 = const.tile([1, 1], f32, tag="ident1")
        nc.vector.memset(ident1, 1.0)

        # Broadcast the (free-dim) biases across 128 partitions via K=1 matmuls.
        bob = persist.tile([P, V_F], f32, tag="bob")
        bvb = persist.tile([P, V_F], f32, tag="bvb")
        with tc.tile_pool(name="ps0", bufs=2, space=PSUM) as ps0:
            for src, dst in ((bo, bob), (bv, bvb)):
                for c in range(V_F // 512):
                    pt = ps0.tile([P, 512], f32, tag="pinit")
                    nc.tensor.matmul(
                        pt, ones_r, src[0:1, c * 512:(c + 1) * 512],
                        start=True, stop=True,
                    )
                    nc.vector.tensor_copy(dst[:, c * 512:(c + 1) * 512], pt)

        # Persistent attention operands + output-projection weights.
        q_sb = [persist.tile([P, HALF], bf16, tag=f"q{m}", name=f"q{m}") for m in range(4)]
        k_sb = [persist.tile([P, L], bf16, tag=f"k{m}", name=f"k{m}") for m in range(4)]
        v_sb = [persist.tile([P, V_F], bf16, tag=f"v{rb}", name=f"v{rb}") for rb in range(L // P)]
        wo_sb = [persist.tile([P, V_F], bf16, tag=f"wo{t}", name=f"wo{t}") for t in range(V_F // P)]
        for t in range(V_F // P):
            nc.sync.dma_start(wo_sb[t], wo_r[t])

        # Staging tiles for the pair exchange (own k^T blocks + own v rows).
        kst = [persist.tile([P, HALF], bf16, tag=f"kst{m}", name=f"kst{m}") for m in range(4)]
        vst = [persist.tile([P, V_F], bf16, tag=f"vst{j}", name=f"vst{j}") for j in range(8)]

        # ---------------- fused projection (own rows only) ----------------
        with (
            tc.tile_pool(name="xp", bufs=1) as xp,
            tc.tile_pool(name="ws", bufs=3) as ws,
            tc.tile_pool(name="psqk", bufs=2, space=PSUM) as psqk,
            tc.tile_pool(name="psv", bufs=1, space=PSUM) as psv,
        ):
            x = []
            for k in range(KT):
                xt = xp.tile([P, HALF], bf16, tag=f"x{k}")
                nc.sync.dma_start(xt, xT_r[k])
                x.append(xt)

            # k_real projection (^T layout) for own rows -> staging + send.
            for m in range(4):
                pk = psqk.tile([P, HALF], f32, tag="psqk")
                for k in range(KT):
                    wsl = ws.tile([P, P], bf16, tag="wk")
                    nc.sync.dma_start(wsl, wqk_r[k, :, (4 + m) * P:(5 + m) * P])
                    for c in range(2):
                        nc.tensor.matmul(
                            pk[:, c * 512:(c + 1) * 512], wsl,
                            x[k][:, c * 512:(c + 1) * 512],
                            start=(k == 0), stop=(k == KT - 1),
                        )
                nc.scalar.activation(kst[m], pk, AF.Identity, bias=bqk[:, 4 + m:5 + m])
                nc.sync.dma_start(ksend_r[m], kst[m])

            # Pairwise exchange of the k^T blocks (ranks are [even, odd] =
            # [global first half, global second half] for both pair members);
            # hidden behind the q and v projections that follow.
            nc.gpsimd.collective_compute(
                "AllGather", mybir.AluOpType.bypass,
                replica_groups=GROUPS,
                ins=[k_send.opt()], outs=[k_recv.opt()],
            )

            # q_real projection (^T layout) -- overlaps the k exchange.
            for m in range(4):
                pq = psqk.tile([P, HALF], f32, tag="psqk")
                for k in range(KT):
                    wsl = ws.tile([P, P], bf16, tag="wq")
                    nc.sync.dma_start(wsl, wqk_r[k, :, m * P:(m + 1) * P])
                    for c in range(2):
                        nc.tensor.matmul(
                            pq[:, c * 512:(c + 1) * 512], wsl,
                            x[k][:, c * 512:(c + 1) * 512],
                            start=(k == 0), stop=(k == KT - 1),
                        )
                nc.scalar.activation(q_sb[m], pq, AF.Identity, bias=bqk[:, m:m + 1])

            # Read back k in global key order (overlaps the v projection).
            for hh in range(2):
                for m in range(4):
                    nc.sync.dma_start(
                        k_sb[m][:, hh * HALF:(hh + 1) * HALF],
                        krecv_r[hh * 4 + m],
                    )

            # v_real|v_imag projection for own rows, row-major -> staging.
            for g in range(2):
                for c in range(2):
                    pv = [psv.tile([P, 512], f32, tag=f"psv{j}", name=f"psv{j}") for j in range(4)]
                    for k in range(KT):
                        wsl = ws.tile([P, 512], bf16, tag="wv")
                        nc.sync.dma_start(wsl, wv_r[k, :, c * 512:(c + 1) * 512])
                        for j in range(4):
                            rl = g * 4 + j
                            nc.tensor.matmul(
                                pv[j], x[k][:, rl * P:(rl + 1) * P], wsl,
                                start=(k == 0), stop=(k == KT - 1),
                            )
                    for j in range(4):
                        rl = g * 4 + j
                        nc.vector.tensor_tensor(
                            vst[rl][:, c * 512:(c + 1) * 512], pv[j],
                            bvb[:, c * 512:(c + 1) * 512], op=ALU.add,
                        )
                for j in range(4):
                    rl = g * 4 + j
                    nc.sync.dma_start(vsend_r[rl], vst[rl])

            # v exchange -- hidden behind the first scores/exp chunk.
            nc.gpsimd.collective_compute(
                "AllGather", mybir.AluOpType.bypass,
                replica_groups=GROUPS,
                ins=[v_send.opt()], outs=[v_recv.opt()],
            )
            for hh in range(2):
                for j in range(8):
                    nc.sync.dma_start(v_sb[hh * 8 + j], vrecv_r[hh * 8 + j])

        # --------------------------- attention ---------------------------
        with (
            tc.tile_pool(name="at", bufs=1) as atp,
            tc.tile_pool(name="zs", bufs=2) as zsp,
            tc.tile_pool(name="ys", bufs=2) as ysp,
            tc.tile_pool(name="sm", bufs=2) as smp,
            tc.tile_pool(name="pssc", bufs=2, space=PSUM) as pssc,
            tc.tile_pool(name="psz", bufs=2, space=PSUM) as psz,
            tc.tile_pool(name="pssum", bufs=1, space=PSUM) as pssum,
            tc.tile_pool(name="pstp", bufs=1, space=PSUM) as pstp,
            tc.tile_pool(name="psy", bufs=1, space=PSUM) as psy,
        ):
            for ch in range(2):
                qs = slice(ch * 512, (ch + 1) * 512)

                # scores^T + masked exp, per key tile
                at = []
                for t in range(L // P):
                    ps = pssc.tile([P, 512], f32, tag="sc")
                    for d in range(4):
                        nc.tensor.matmul(
                            ps, k_sb[d][:, t * P:(t + 1) * P], q_sb[d][:, qs],
                            start=(d == 0), stop=(d == 3),
                        )
                    a = atp.tile([P, 512], bf16, tag=f"at{t}")
                    nc.scalar.activation(
                        a, ps, AF.Exp, bias=mb[:, t:t + 1], scale=SCALE
                    )
                    at.append(a)

                # softmax row-sums: ones^T @ attn^T, accumulated over key tiles
                sp = pssum.tile([1, 512], f32, tag="sum")
                for t in range(L // P):
                    nc.tensor.matmul(
                        sp, ones_c, at[t], start=(t == 0), stop=(t == L // P - 1)
                    )
                sums = smp.tile([1, 512], f32, tag="sums")
                nc.vector.tensor_copy(sums, sp)
                rc = []
                for s in range(4):
                    tp = pstp.tile([P, 1], f32, tag="tp")
                    nc.tensor.transpose(tp, sums[0:1, s * P:(s + 1) * P], ident1)
                    r = smp.tile([P, 1], f32, tag=f"rc{s}", name=f"rc{s}")
                    nc.vector.reciprocal(r, tp)
                    rc.append(r)

                # Z^T = sum_key v[key, dblk] * attn^T[key, q]
                z_sb = []
                for dblk in range(V_F // P):
                    pz = psz.tile([P, 512], f32, tag="z")
                    for t in range(L // P):
                        nc.tensor.matmul(
                            pz, v_sb[t][:, dblk * P:(dblk + 1) * P], at[t],
                            start=(t == 0), stop=(t == L // P - 1),
                        )
                    z = zsp.tile([P, 512], bf16, tag=f"z{dblk}", name=f"z{dblk}")
                    nc.vector.tensor_copy(z, pz)
                    z_sb.append(z)

                # output projection + deferred normalization + bias
                for s in range(4):
                    py = psy.tile([P, V_F], f32, tag="y")
                    for nch in range(2):
                        for dblk in range(V_F // P):
                            nc.tensor.matmul(
                                py[:, nch * 512:(nch + 1) * 512],
                                z_sb[dblk][:, s * P:(s + 1) * P],
                                wo_sb[dblk][:, nch * 512:(nch + 1) * 512],
                                start=(dblk == 0), stop=(dblk == V_F // P - 1),
                            )
                    ysb = ysp.tile([P, V_F], f32, tag="ysb")
                    nc.vector.scalar_tensor_tensor(
                        ysb, py, rc[s], bob, op0=ALU.mult, op1=ALU.add
                    )
                    r0 = ch * 512 + s * P
                    nc.sync.dma_start(y_d[r0:r0 + P, :], ysb)

    with tile.TileContext(nc) as tc:
        for r in range(reps):
            if r:
                tc.strict_bb_all_engine_barrier()
            with ExitStack() as ctx:
                _emit_body(tc, ctx)

    nc.compile()
    return nc


def get_nc(reps=1):
    key = f"nc{reps}"
    if key not in _NC_CACHE:
        _NC_CACHE[key] = _build_program(reps)
    return _NC_CACHE[key]


def prepare_in_maps(inputs):
    bf = ml_dtypes.bfloat16
    f32 = np.float32

    q_real = np.asarray(inputs["q_real"], f32)
    q_imag = np.asarray(inputs["q_imag"], f32)
    k_real = np.asarray(inputs["k_real"], f32)
    k_imag = np.asarray(inputs["k_imag"], f32)
    v_real = np.asarray(inputs["v_real"], f32)
    v_imag = np.asarray(inputs["v_imag"], f32)
    pad_mask = np.asarray(inputs["pad_mask"]).astype(bool)
    W_qkv = np.asarray(inputs["W_qkv"], f32)
    b_qkv = np.asarray(inputs["b_qkv"], f32)
    W_out = np.asarray(inputs["W_out"], f32)
    b_out = np.asarray(inputs["b_out"], f32)

    sel_qk = np.r_[0:D, 2 * D:3 * D]          # q_real + k_real output blocks
    wqkT = np.ascontiguousarray(W_qkv[sel_qk, :].T.astype(bf))
    wvT = np.ascontiguousarray(W_qkv[4 * D:6 * D, :].T.astype(bf))
    woT = np.ascontiguousarray(W_out.T.astype(bf))
    bqk = np.ascontiguousarray(b_qkv[sel_qk].reshape(QK_F // P, P).T.astype(f32))
    bv_row = np.ascontiguousarray(b_qkv[4 * D:6 * D].reshape(1, V_F).astype(f32))
    bo_row = np.ascontiguousarray(b_out.reshape(1, V_F).astype(f32))

    x = np.concatenate([q_real, q_imag, k_real, k_imag, v_real, v_imag], axis=-1)

    in_maps = []
    for c in range(NCORES):
        b, h = divmod(c, 2)
        xT = np.ascontiguousarray(x[b][h * HALF:(h + 1) * HALF].T.astype(bf))
        mbias = np.where(pad_mask[b], f32(NEG), f32(0.0))
        mbt = np.ascontiguousarray(mbias.reshape(L // P, P).T.astype(f32))
        in_maps.append({
            "xT": xT, "wqkT": wqkT, "wvT": wvT, "woT": woT,
            "maskb": mbt, "bqk": bqk, "bv": bv_row, "bo": bo_row,
        })
    return in_maps


def assemble_outputs(results):
    out_real = np.empty((B, L, D), np.float32)
    out_imag = np.empty((B, L, D), np.float32)
    for c in range(NCORES):
        y = np.asarray(results[c]["y"], np.float32)
        b, h = divmod(c, 2)
        out_real[b, h * HALF:(h + 1) * HALF] = y[:, :D]
        out_imag[b, h * HALF:(h + 1) * HALF] = y[:, D:]
    return out_real, out_imag


def _make_executor(reps=1):
    """One jitted SPMD callable per process (mirrors bass2jax.run_bass_via_pjrt
    but is built once and reused, so repeated runs don't recompile)."""
    import jax
    from concourse import bass2jax, mybir

    try:
        jax.config.update("jax_compilation_cache_dir", "/tmp/jax_neff_cache")
        jax.config.update("jax_persistent_cache_min_compile_time_secs", 5.0)
    except Exception:
        pass

    nc = get_nc(reps)
    bass2jax.install_neuronx_cc_hook()
    partition_name = nc.partition_id_tensor.name if nc.partition_id_tensor else None

    in_names, out_names, out_avals, zero_outs = [], [], [], []
    for alloc in nc.m.functions[0].allocations:
        if not isinstance(alloc, mybir.MemoryLocationSet):
            continue
        name = alloc.memorylocations[0].name
        if alloc.kind == "ExternalInput":
            if name != partition_name:
                in_names.append(name)
        elif alloc.kind == "ExternalOutput":
            out_names.append(name)
            shape = tuple(alloc.tensor_shape)
            dtype = mybir.dt.np(alloc.dtype)
            out_avals.append(jax.core.ShapedArray(shape, dtype))
            zero_outs.append((shape, dtype))
    n_params = len(in_names)
    n_outs = len(out_avals)
    all_in_names = list(in_names) + list(out_names)
    if partition_name is not None:
        all_in_names.append(partition_name)

    def _body(*args):
        operands = list(args)
        if partition_name is not None:
            operands.append(bass2jax.partition_id_tensor())
        outs = bass2jax._bass_exec_p.bind(
            *operands,
            out_avals=tuple(out_avals),
            in_names=tuple(all_in_names),
            out_names=tuple(out_names),
            lowering_input_output_aliases=(),
            sim_require_finite=True,
            sim_require_nnan=True,
            nc=nc,
        )
        return tuple(outs)

    devices = jax.devices()[:NCORES]
    assert len(devices) == NCORES
    mesh = bass2jax.Mesh(np.asarray(devices), ("core",))
    in_specs = (bass2jax.PartitionSpec("core"),) * (n_params + n_outs)
    out_specs = (bass2jax.PartitionSpec("core"),) * n_outs
    donate = tuple(range(n_params, n_params + n_outs))
    sharded = jax.jit(
        bass2jax.shard_map(
            _body, mesh=mesh, in_specs=in_specs,
            out_specs=out_specs, check_rep=False,
        ),
        donate_argnums=donate,
        keep_unused=True,
    )
    return {
        "sharded": sharded,
        "mesh": mesh,
        "in_names": in_names,
        "out_names": out_names,
        "out_avals": out_avals,
        "zero_outs": zero_outs,
    }


def get_executor(reps=1):
    key = f"exec{reps}"
    if key not in _NC_CACHE:
        _NC_CACHE[key] = _make_executor(reps)
    return _NC_CACHE[key]


def concat_inputs(in_maps, ex):
    return [
        np.concatenate([np.asarray(in_maps[c][n]) for c in range(NCORES)], axis=0)
        for n in ex["in_names"]
    ]


def make_zero_outs(ex):
    return [
        np.zeros((NCORES * s[0], *s[1:]), d) for (s, d) in ex["zero_outs"]
    ]


def execute(concat_in, ex):
    out_arrs = ex["sharded"](*concat_in, *make_zero_outs(ex))
    results = [
        {
            name: np.asarray(out_arrs[i]).reshape(
                NCORES, *ex["out_avals"][i].shape
            )[c]
            for i, name in enumerate(ex["out_names"])
        }
        for c in range(NCORES)
    ]
    return results


def run(inputs, trace=False):
    from concourse.bass_utils import run_bass_kernel_spmd

    nc = get_nc()
    in_maps = prepare_in_maps(inputs)
    return run_bass_kernel_spmd(
        nc, in_maps, core_ids=list(range(NCORES)), trace=trace
    )


def kernel(**inputs):
    ex = get_executor()
    in_maps = prepare_in_maps(inputs)
    results = execute(concat_inputs(in_maps, ex), ex)
    return assemble_outputs(results)
